# revision 23
# baseline (speedup 1.0000x reference)
"""Content-based addressing read (DNC-style) for Trainium2 — v9.

Computes softmax_n( strengths[r] * cos_sim(memory[b,n,:], read_vectors[b,:,r]) )
for B=16, N=32768, W=128, R=8, sharded batch-parallel across 8 NeuronCores
(2 batches per core).

Design (v9 = v5 + dual DMA queues + deeper input pool):
  - memory pre-transposed on the HOST to memT[b, w, n] and cast to bf16:
    the PE never transposes, DMA traffic halves.  Mem groups (1MB) stream
    on two queues (sync=even, gpsimd=odd; gpsimd runs no compute so its
    queue is a pure DMA pacer); prep/out DMAs use the scalar queue.
  - sim: per 128-n tile, stationary = memT tile (bf16), moving = rvp
    (128x8 bf16) -> PSUM lands (n-on-partitions, r) directly.
  - row norms: square memT (ACT/DVE split, per half-group; `square` is in
    every ACT table so it never forces a table load), then per-tile matmul
    stationary = sq tile, moving = ones column -> norm^2 in PSUM.  Norm
    matmuls pipeline HALF a group behind the sims, so the drain-to-tail is
    only 16 matmuls.
  - per group: DVE reciprocal on norm PSUM + ACT Sqrt -> inv_nrm, DVE
    multiplies sim-PSUM x inv_nrm into scores (128, R, T).
  - one early ACT Exp over groups 0..5 once group 5 is normalized (3 ACT
    table loads per batch total); tail handles groups 6-7 only.
  - output scaled into bf16 tiles (halves the store traffic); host upcasts.
  - softmax numerics as baseline: no max subtraction (|scores| <= 1), no
    +1e-8 (normalizer ~128 makes it an fp32 no-op).

Output in DRAM is (b, o, p, r, t') bf16 with n = (o*T/2 + t')*128 + p; host
re-transposes and upcasts.
"""

import sys

for _p in ("/opt/trn_rl_repo",):
    if _p not in sys.path:
        sys.path.insert(0, _p)

from contextlib import ExitStack

import numpy as np
import ml_dtypes

import concourse.bass as bass
import concourse.bacc as bacc
import concourse.tile as tile
from concourse import mybir
from concourse.bass_utils import run_bass_kernel_spmd

F32 = mybir.dt.float32
BF16 = mybir.dt.bfloat16
F8 = mybir.dt.float8e4
AF = mybir.ActivationFunctionType

B, N, W, R = 16, 32768, 128, 8
NCORES = 8
BLOC = B // NCORES          # batches per core
T = N // 128                # 256 n-tiles of 128 per batch
NG = 8                      # DMA groups per batch
TPG = T // NG               # 32 tiles per group (4096 n, 1MB bf16)
SUB = 2                     # norm-pipeline sub-slots per group
PIPE_LAG = 0                # sub-slots the norm matmuls trail the sims by
TPS = TPG // SUB            # 16 tiles per sub-slot

# ---- tuning knobs ----
# engine squaring each half-group (cycled): "v"=DVE, "a"=ACT
SQUARE_ENGINES = "av"
# DMA queue per group (cycled): "s"=sync, "g"=gpsimd
DMA_QUEUES = "s"
EARLY_G = NG - 3            # groups [0..EARLY_G] exponentiated early
OUT_SPLIT = 2               # final scale+store chunks (tail overlap)
CH = T // OUT_SPLIT


def build_program():
    nc = bacc.Bacc("TRN2", target_bir_lowering=False, debug=False, num_devices=NCORES)

    memT = nc.dram_tensor("memT", [BLOC, W, N], BF16, kind="ExternalInput").ap()
    rv = nc.dram_tensor("read_vectors", [BLOC, W, R], F32, kind="ExternalInput").ap()
    rs = nc.dram_tensor("read_strengths", [BLOC, R], F32, kind="ExternalInput").ap()
    ones = nc.dram_tensor("ones", [128, 128], F32, kind="ExternalInput").ap()
    out = nc.dram_tensor(
        "out", [BLOC, OUT_SPLIT, 128, R, CH], BF16, kind="ExternalOutput"
    ).ap()

    with ExitStack() as ctx:
        tc = ctx.enter_context(tile.TileContext(nc))

        const_pool = ctx.enter_context(tc.tile_pool(name="const", bufs=1))
        ones_t = const_pool.tile([128, 128], F32)
        nc.scalar.dma_start(ones_t[:], ones)
        ones1_bf = const_pool.tile([128, 1], BF16)
        nc.vector.tensor_copy(ones1_bf[:], ones_t[:, 0:1])
        # warm the SQRT act table while the first DMAs stream
        sqrt_warm = const_pool.tile([128, 1], F32)
        nc.scalar.activation(sqrt_warm[:], ones_t[:, 0:1], AF.Sqrt)

        in_pool = ctx.enter_context(tc.tile_pool(name="mem_in", bufs=6))
        sq_pool = ctx.enter_context(tc.tile_pool(name="sq", bufs=4))
        scps_pool = ctx.enter_context(tc.tile_pool(name="scps", bufs=4, space="PSUM"))
        nrps_pool = ctx.enter_context(tc.tile_pool(name="nrps", bufs=3, space="PSUM"))
        prep_pool = ctx.enter_context(tc.tile_pool(name="prep", bufs=1, space="PSUM"))
        smalls = ctx.enter_context(tc.tile_pool(name="smalls", bufs=3))
        rvp_pool = ctx.enter_context(tc.tile_pool(name="rvps", bufs=1))
        score_pool = ctx.enter_context(tc.tile_pool(name="scores", bufs=2))
        outbf_pool = ctx.enter_context(tc.tile_pool(name="outbf", bufs=2))

        # ---- read-vector prep for both batches: rv' = rv*strength/||rv|| ----
        rvp_bfs = []
        for b in range(BLOC):
            rv_t = smalls.tile([128, R], F32)
            nc.scalar.dma_start(rv_t[:], rv[b])
            rs_t = smalls.tile([1, R], F32)
            nc.scalar.dma_start(rs_t[:], rs[b : b + 1, :])

            rv2 = smalls.tile([128, R], F32)
            nc.vector.tensor_mul(rv2[:], rv_t[:], rv_t[:])
            nv2_ps = prep_pool.tile([128, R], F32, tag="prep")
            nc.tensor.matmul(nv2_ps[:], ones_t[:], rv2[:], start=True, stop=True)
            rnv = smalls.tile([128, R], F32)
            nc.vector.reciprocal(rnv[:], nv2_ps[:])
            inv_nv = smalls.tile([128, R], F32)
            nc.scalar.activation(inv_nv[:], rnv[:], AF.Sqrt)
            rsb_ps = prep_pool.tile([128, R], F32, tag="prep")
            nc.tensor.matmul(
                rsb_ps[:], ones_t[0:1, :], rs_t[:], start=True, stop=True
            )
            factor = smalls.tile([128, R], F32)
            nc.vector.tensor_mul(factor[:], rsb_ps[:], inv_nv[:])
            rvp = smalls.tile([128, R], F32, tag="rvp")
            nc.vector.tensor_mul(rvp[:], rv_t[:], factor[:])
            rvp_bf = rvp_pool.tile([128, R], BF16, tag=f"rvpbf{b}")
            nc.vector.tensor_copy(rvp_bf[:], rvp[:])
            rvp_bfs.append(rvp_bf)

        sq_i = 0
        subs = []  # pipeline: (g, s, sq_g, scps, nrps, scores, s1a_holder)

        def issue_norm_sub(ent):
            g, s, sq_g, scps, nrps, scores, s1a = ent
            for j in range(s * TPS, (s + 1) * TPS):
                nc.tensor.matmul(
                    nrps[:, j : j + 1],
                    sq_g[:, j * 128 : (j + 1) * 128],
                    ones1_bf[:],
                    start=True,
                    stop=True,
                )
            if s < SUB - 1:
                return
            # group complete: inv_nrm = 1/sqrt(norm^2), then scale sim PSUM
            nrm_g = smalls.tile([128, TPG], F32, tag="nrm")
            nc.scalar.activation(nrm_g[:], nrps[:], AF.Sqrt)
            inv_nrm = smalls.tile([128, TPG], F32, tag="invnrm")
            nc.vector.reciprocal(inv_nrm[:], nrm_g[:])
            nc.vector.tensor_mul(
                scores[:, :, g * TPG : (g + 1) * TPG],
                scps[:].rearrange("p (t r) -> p t r", r=R).transpose([0, 2, 1]),
                inv_nrm[:].unsqueeze(1).broadcast_to([128, R, TPG]),
            )
            if g == EARLY_G:
                hi = (EARLY_G + 1) * TPG
                nc.scalar.activation(
                    scores[:, :, :hi], scores[:, :, :hi], AF.Exp
                )
                nc.vector.reduce_sum(
                    s1a[:], scores[:, :, :hi], axis=mybir.AxisListType.X
                )

        for b in range(BLOC):
            scores = score_pool.tile([128, R, T], F32)
            s1a = smalls.tile([128, R], F32, tag="s1a")
            rvp_bf = rvp_bfs[b]

            for g in range(NG):
                mem_g = in_pool.tile([128, TPG * 128], BF16)
                qe = DMA_QUEUES[g % len(DMA_QUEUES)]
                src = memT[b, :, g * TPG * 128 : (g + 1) * TPG * 128]
                if qe == "g":
                    nc.gpsimd.dma_start(mem_g[:], src)
                else:
                    nc.sync.dma_start(mem_g[:], src)

                # squares for row norms, issued per half-group
                sq_g = sq_pool.tile([128, TPG * 128], BF16)
                scps = scps_pool.tile([128, TPG * R], F32)
                nrps = nrps_pool.tile([128, TPG], F32)
                for s in range(SUB):
                    ssl = slice(s * TPS * 128, (s + 1) * TPS * 128)
                    se = SQUARE_ENGINES[sq_i % len(SQUARE_ENGINES)]
                    sq_i += 1
                    if se == "a":
                        nc.scalar.square(sq_g[:, ssl], mem_g[:, ssl])
                    elif se == "g":
                        nc.gpsimd.tensor_mul(
                            sq_g[:, ssl], mem_g[:, ssl], mem_g[:, ssl]
                        )
                    else:
                        nc.vector.tensor_mul(sq_g[:, ssl], mem_g[:, ssl], mem_g[:, ssl])

                for s in range(SUB):
                    for j in range(s * TPS, (s + 1) * TPS):
                        nc.tensor.matmul(
                            scps[:, j * R : (j + 1) * R],
                            mem_g[:, j * 128 : (j + 1) * 128],
                            rvp_bf[:],
                            start=True,
                            stop=True,
                        )
                    subs.append((g, s, sq_g, scps, nrps, scores, s1a))
                    if len(subs) > PIPE_LAG:
                        issue_norm_sub(subs.pop(0))

            # flush before this batch's softmax tail reads `scores`
            while subs:
                issue_norm_sub(subs.pop(0))

            # ---- softmax tail (groups EARLY_G+1 .. NG-1) ----
            lo = (EARLY_G + 1) * TPG
            nc.scalar.activation(scores[:, :, lo:], scores[:, :, lo:], AF.Exp)
            s1 = smalls.tile([128, R], F32, tag="s1")
            nc.vector.reduce_sum(s1[:], scores[:, :, lo:], axis=mybir.AxisListType.X)
            if EARLY_G >= 0:
                nc.vector.tensor_add(s1[:], s1[:], s1a[:])
            tot_ps = prep_pool.tile([128, R], F32, tag="prep")
            nc.tensor.matmul(tot_ps[:], ones_t[:], s1[:], start=True, stop=True)
            inv_tot = smalls.tile([128, R], F32)
            nc.vector.reciprocal(inv_tot[:], tot_ps[:])
            for o in range(OUT_SPLIT):
                sl = slice(o * CH, (o + 1) * CH)
                ob = outbf_pool.tile([128, R, CH], BF16)
                nc.vector.tensor_mul(
                    ob[:],
                    scores[:, :, sl],
                    inv_tot[:].unsqueeze(2).broadcast_to([128, R, CH]),
                )
                nc.scalar.dma_start(out[b, o], ob[:])

    nc.compile()
    return nc


_program = None
last_results = None


def _get_program():
    global _program
    if _program is None:
        _program = build_program()
    return _program


def kernel(memory, read_strengths, read_vectors):
    memory = np.asarray(memory, dtype=np.float32)
    read_strengths = np.asarray(read_strengths, dtype=np.float32)
    read_vectors = np.asarray(read_vectors, dtype=np.float32)

    nc = _get_program()
    ones_m = np.ones((128, 128), dtype=np.float32)
    in_maps = []
    for c in range(NCORES):
        sl = slice(c * BLOC, (c + 1) * BLOC)
        memT = np.ascontiguousarray(memory[sl].transpose(0, 2, 1)).astype(
            ml_dtypes.bfloat16
        )
        in_maps.append(
            {
                "memT": memT,
                "read_vectors": np.ascontiguousarray(read_vectors[sl]),
                "read_strengths": np.ascontiguousarray(read_strengths[sl]),
                "ones": ones_m,
            }
        )

    global last_results
    last_results = run_bass_kernel_spmd(nc, in_maps, list(range(NCORES)))
    res = last_results.results
    outs = []
    for c in range(NCORES):
        o = np.asarray(res[c]["out"]).astype(np.float32)
        # (BLOC, OUT_SPLIT, 128, R, CH); n = (o*CH + t')*128 + p
        outs.append(o.transpose(0, 1, 4, 2, 3).reshape(BLOC, N, R))
    return np.concatenate(outs, axis=0)


# revision 24
# speedup vs baseline: 1.0371x; 1.0371x over previous
"""Content-based addressing read (DNC-style) for Trainium2.

Computes softmax_n( strengths[r] * cos_sim(memory[b,n,:], read_vectors[b,:,r]) )
for B=16, N=32768, W=128, R=8, sharded batch-parallel across 8 NeuronCores
(2 batches per core).

Design:
  - memory pre-transposed on the HOST to memT[b, w, n] and cast to bf16:
    the PE never transposes, DMA traffic halves.  Mem groups (1MB) stream
    gaplessly on the sync queue (~360 GB/s); prep/out DMAs use the scalar
    queue.
  - sim: per 128-n tile, stationary = memT tile (bf16), moving = rvp
    (128x8 bf16) -> PSUM lands (n-on-partitions, r) directly.
  - row norms: square memT (ACT/DVE split, per half-group; `square` is in
    every ACT table so it never forces a table load), then per-tile matmul
    stationary = sq tile, moving = ones column -> norm^2 in PSUM.  Norm
    matmuls pipeline HALF a group behind the sims, so the drain-to-tail is
    only 16 matmuls.
  - per group: ACT Sqrt on the norm PSUM, then DVE reciprocal + multiply
    (back-to-back on one engine, minimizing cross-engine handoffs) scales
    the sim PSUM into scores (128, R, T).
  - one early ACT Exp over groups 0..5 once group 5 is normalized (3 ACT
    table loads per batch total); tail handles groups 6-7 only.
  - output scaled into bf16 tiles (halves the store traffic); host upcasts.
  - softmax numerics as baseline: no max subtraction (|scores| <= 1), no
    +1e-8 (normalizer ~128 makes it an fp32 no-op).

Output in DRAM is (b, o, p, r, t') bf16 with n = (o*T/2 + t')*128 + p; host
re-transposes and upcasts.
"""

import sys

for _p in ("/opt/trn_rl_repo",):
    if _p not in sys.path:
        sys.path.insert(0, _p)

from contextlib import ExitStack

import numpy as np
import ml_dtypes

import concourse.bass as bass
import concourse.bacc as bacc
import concourse.tile as tile
from concourse import mybir
from concourse.bass_utils import run_bass_kernel_spmd

F32 = mybir.dt.float32
BF16 = mybir.dt.bfloat16
F8 = mybir.dt.float8e4
AF = mybir.ActivationFunctionType

B, N, W, R = 16, 32768, 128, 8
NCORES = 8
BLOC = B // NCORES          # batches per core
T = N // 128                # 256 n-tiles of 128 per batch
NG = 8                      # DMA groups per batch
TPG = T // NG               # 32 tiles per group (4096 n, 1MB bf16)
SUB = 2                     # norm-pipeline sub-slots per group
PIPE_LAG = 1                # sub-slots the norm matmuls trail the sims by
TPS = TPG // SUB            # 16 tiles per sub-slot

# ---- tuning knobs ----
# engine squaring each half-group (cycled): "v"=DVE, "a"=ACT
SQUARE_ENGINES = "av"
# DMA queue per group (cycled): "s"=sync, "g"=gpsimd
DMA_QUEUES = "s"
EARLY_G = NG - 3            # groups [0..EARLY_G] exponentiated early
OUT_SPLIT = 2               # final scale+store chunks (tail overlap)
CH = T // OUT_SPLIT


def build_program():
    nc = bacc.Bacc("TRN2", target_bir_lowering=False, debug=False, num_devices=NCORES)

    memT = nc.dram_tensor("memT", [BLOC, W, N], BF16, kind="ExternalInput").ap()
    rv = nc.dram_tensor("read_vectors", [BLOC, W, R], F32, kind="ExternalInput").ap()
    rs = nc.dram_tensor("read_strengths", [BLOC, R], F32, kind="ExternalInput").ap()
    ones = nc.dram_tensor("ones", [128, 128], F32, kind="ExternalInput").ap()
    out = nc.dram_tensor(
        "out", [BLOC, OUT_SPLIT, 128, R, CH], BF16, kind="ExternalOutput"
    ).ap()

    with ExitStack() as ctx:
        tc = ctx.enter_context(tile.TileContext(nc))

        const_pool = ctx.enter_context(tc.tile_pool(name="const", bufs=1))
        ones_t = const_pool.tile([128, 128], F32)
        nc.scalar.dma_start(ones_t[:], ones)
        ones1_bf = const_pool.tile([128, 1], BF16)
        nc.vector.tensor_copy(ones1_bf[:], ones_t[:, 0:1])
        # warm the SQRT act table while the first DMAs stream
        sqrt_warm = const_pool.tile([128, 1], F32)
        nc.scalar.activation(sqrt_warm[:], ones_t[:, 0:1], AF.Sqrt)

        in_pool = ctx.enter_context(tc.tile_pool(name="mem_in", bufs=6))
        sq_pool = ctx.enter_context(tc.tile_pool(name="sq", bufs=4))
        scps_pool = ctx.enter_context(tc.tile_pool(name="scps", bufs=4, space="PSUM"))
        nrps_pool = ctx.enter_context(tc.tile_pool(name="nrps", bufs=3, space="PSUM"))
        prep_pool = ctx.enter_context(tc.tile_pool(name="prep", bufs=1, space="PSUM"))
        smalls = ctx.enter_context(tc.tile_pool(name="smalls", bufs=3))
        rvp_pool = ctx.enter_context(tc.tile_pool(name="rvps", bufs=1))
        score_pool = ctx.enter_context(tc.tile_pool(name="scores", bufs=2))
        outbf_pool = ctx.enter_context(tc.tile_pool(name="outbf", bufs=2))

        # ---- read-vector prep for both batches: rv' = rv*strength/||rv|| ----
        rvp_bfs = []
        for b in range(BLOC):
            rv_t = smalls.tile([128, R], F32)
            nc.scalar.dma_start(rv_t[:], rv[b])
            rs_t = smalls.tile([1, R], F32)
            nc.scalar.dma_start(rs_t[:], rs[b : b + 1, :])

            rv2 = smalls.tile([128, R], F32)
            nc.vector.tensor_mul(rv2[:], rv_t[:], rv_t[:])
            nv2_ps = prep_pool.tile([128, R], F32, tag="prep")
            nc.tensor.matmul(nv2_ps[:], ones_t[:], rv2[:], start=True, stop=True)
            rnv = smalls.tile([128, R], F32)
            nc.vector.reciprocal(rnv[:], nv2_ps[:])
            inv_nv = smalls.tile([128, R], F32)
            nc.scalar.activation(inv_nv[:], rnv[:], AF.Sqrt)
            rsb_ps = prep_pool.tile([128, R], F32, tag="prep")
            nc.tensor.matmul(
                rsb_ps[:], ones_t[0:1, :], rs_t[:], start=True, stop=True
            )
            factor = smalls.tile([128, R], F32)
            nc.vector.tensor_mul(factor[:], rsb_ps[:], inv_nv[:])
            rvp = smalls.tile([128, R], F32, tag="rvp")
            nc.vector.tensor_mul(rvp[:], rv_t[:], factor[:])
            rvp_bf = rvp_pool.tile([128, R], BF16, tag=f"rvpbf{b}")
            nc.vector.tensor_copy(rvp_bf[:], rvp[:])
            rvp_bfs.append(rvp_bf)

        sq_i = 0
        subs = []  # pipeline: (g, s, sq_g, scps, nrps, scores, s1a_holder)

        def issue_norm_sub(ent):
            g, s, sq_g, scps, nrps, scores, s1a = ent
            for j in range(s * TPS, (s + 1) * TPS):
                nc.tensor.matmul(
                    nrps[:, j : j + 1],
                    sq_g[:, j * 128 : (j + 1) * 128],
                    ones1_bf[:],
                    start=True,
                    stop=True,
                )
            if s < SUB - 1:
                return
            # group complete: inv_nrm = 1/sqrt(norm^2), then scale sim PSUM
            nrm_g = smalls.tile([128, TPG], F32, tag="nrm")
            nc.scalar.activation(nrm_g[:], nrps[:], AF.Sqrt)
            inv_nrm = smalls.tile([128, TPG], F32, tag="invnrm")
            nc.vector.reciprocal(inv_nrm[:], nrm_g[:])
            nc.vector.tensor_mul(
                scores[:, :, g * TPG : (g + 1) * TPG],
                scps[:].rearrange("p (t r) -> p t r", r=R).transpose([0, 2, 1]),
                inv_nrm[:].unsqueeze(1).broadcast_to([128, R, TPG]),
            )
            if g == EARLY_G:
                hi = (EARLY_G + 1) * TPG
                nc.scalar.activation(
                    scores[:, :, :hi], scores[:, :, :hi], AF.Exp
                )
                nc.vector.reduce_sum(
                    s1a[:], scores[:, :, :hi], axis=mybir.AxisListType.X
                )

        for b in range(BLOC):
            scores = score_pool.tile([128, R, T], F32)
            s1a = smalls.tile([128, R], F32, tag="s1a")
            rvp_bf = rvp_bfs[b]

            for g in range(NG):
                mem_g = in_pool.tile([128, TPG * 128], BF16)
                qe = DMA_QUEUES[g % len(DMA_QUEUES)]
                src = memT[b, :, g * TPG * 128 : (g + 1) * TPG * 128]
                if qe == "g":
                    nc.gpsimd.dma_start(mem_g[:], src)
                else:
                    nc.sync.dma_start(mem_g[:], src)

                # squares for row norms, issued per half-group
                sq_g = sq_pool.tile([128, TPG * 128], BF16)
                scps = scps_pool.tile([128, TPG * R], F32)
                nrps = nrps_pool.tile([128, TPG], F32)
                for s in range(SUB):
                    ssl = slice(s * TPS * 128, (s + 1) * TPS * 128)
                    se = SQUARE_ENGINES[sq_i % len(SQUARE_ENGINES)]
                    sq_i += 1
                    if se == "a":
                        nc.scalar.square(sq_g[:, ssl], mem_g[:, ssl])
                    elif se == "g":
                        nc.gpsimd.tensor_mul(
                            sq_g[:, ssl], mem_g[:, ssl], mem_g[:, ssl]
                        )
                    else:
                        nc.vector.tensor_mul(sq_g[:, ssl], mem_g[:, ssl], mem_g[:, ssl])

                for s in range(SUB):
                    for j in range(s * TPS, (s + 1) * TPS):
                        nc.tensor.matmul(
                            scps[:, j * R : (j + 1) * R],
                            mem_g[:, j * 128 : (j + 1) * 128],
                            rvp_bf[:],
                            start=True,
                            stop=True,
                        )
                    subs.append((g, s, sq_g, scps, nrps, scores, s1a))
                    if len(subs) > PIPE_LAG:
                        issue_norm_sub(subs.pop(0))

            # flush before this batch's softmax tail reads `scores`
            while subs:
                issue_norm_sub(subs.pop(0))

            # ---- softmax tail (groups EARLY_G+1 .. NG-1) ----
            lo = (EARLY_G + 1) * TPG
            nc.scalar.activation(scores[:, :, lo:], scores[:, :, lo:], AF.Exp)
            s1 = smalls.tile([128, R], F32, tag="s1")
            nc.vector.reduce_sum(s1[:], scores[:, :, lo:], axis=mybir.AxisListType.X)
            if EARLY_G >= 0:
                nc.vector.tensor_add(s1[:], s1[:], s1a[:])
            tot_ps = prep_pool.tile([128, R], F32, tag="prep")
            nc.tensor.matmul(tot_ps[:], ones_t[:], s1[:], start=True, stop=True)
            inv_tot = smalls.tile([128, R], F32)
            nc.vector.reciprocal(inv_tot[:], tot_ps[:])
            for o in range(OUT_SPLIT):
                sl = slice(o * CH, (o + 1) * CH)
                ob = outbf_pool.tile([128, R, CH], BF16)
                nc.vector.tensor_mul(
                    ob[:],
                    scores[:, :, sl],
                    inv_tot[:].unsqueeze(2).broadcast_to([128, R, CH]),
                )
                nc.scalar.dma_start(out[b, o], ob[:])

    nc.compile()
    return nc


_program = None
last_results = None


def _get_program():
    global _program
    if _program is None:
        _program = build_program()
    return _program


def kernel(memory, read_strengths, read_vectors):
    memory = np.asarray(memory, dtype=np.float32)
    read_strengths = np.asarray(read_strengths, dtype=np.float32)
    read_vectors = np.asarray(read_vectors, dtype=np.float32)

    nc = _get_program()
    ones_m = np.ones((128, 128), dtype=np.float32)
    in_maps = []
    for c in range(NCORES):
        sl = slice(c * BLOC, (c + 1) * BLOC)
        memT = np.ascontiguousarray(memory[sl].transpose(0, 2, 1)).astype(
            ml_dtypes.bfloat16
        )
        in_maps.append(
            {
                "memT": memT,
                "read_vectors": np.ascontiguousarray(read_vectors[sl]),
                "read_strengths": np.ascontiguousarray(read_strengths[sl]),
                "ones": ones_m,
            }
        )

    global last_results
    last_results = run_bass_kernel_spmd(nc, in_maps, list(range(NCORES)))
    res = last_results.results
    outs = []
    for c in range(NCORES):
        o = np.asarray(res[c]["out"]).astype(np.float32)
        # (BLOC, OUT_SPLIT, 128, R, CH); n = (o*CH + t')*128 + p
        outs.append(o.transpose(0, 1, 4, 2, 3).reshape(BLOC, N, R))
    return np.concatenate(outs, axis=0)


# revision 25
# speedup vs baseline: 1.0893x; 1.0503x over previous
"""Content-based addressing read (DNC-style) for Trainium2.

Computes softmax_n( strengths[r] * cos_sim(memory[b,n,:], read_vectors[b,:,r]) )
for B=16, N=32768, W=128, R=8, sharded batch-parallel across 8 NeuronCores
(2 batches per core).

Design:
  - memory pre-transposed on the HOST to memT[b, w, n] and cast to bf16:
    the PE never transposes, DMA traffic halves.  Mem groups (1MB) stream
    gaplessly on the sync queue (~360 GB/s); prep/out DMAs use the scalar
    queue.
  - sim: per 128-n tile, stationary = memT tile (bf16), moving = rvp
    (128x8 bf16) -> PSUM lands (n-on-partitions, r) directly.
  - row norms: square memT (ACT/DVE split, per half-group; `square` is in
    every ACT table so it never forces a table load), then per-tile matmul
    stationary = sq tile, moving = ones column -> norm^2 in PSUM.  Norm
    matmuls pipeline HALF a group behind the sims, so the drain-to-tail is
    only 16 matmuls.
  - per group: ACT Sqrt on the norm PSUM, then DVE reciprocal + multiply
    (back-to-back on one engine, minimizing cross-engine handoffs) scales
    the sim PSUM into scores (128, R, T).
  - one early ACT Exp over groups 0..5 once group 5 is normalized (3 ACT
    table loads per batch total); tail handles groups 6-7 only.
  - output scaled into bf16 tiles (halves the store traffic); host upcasts.
  - softmax numerics as baseline: no max subtraction (|scores| <= 1), no
    +1e-8 (normalizer ~128 makes it an fp32 no-op).

Output in DRAM is (b, o, p, r, t') bf16 with n = (o*T/2 + t')*128 + p; host
re-transposes and upcasts.
"""

import sys

for _p in ("/opt/trn_rl_repo",):
    if _p not in sys.path:
        sys.path.insert(0, _p)

from contextlib import ExitStack

import numpy as np
import ml_dtypes

import concourse.bass as bass
import concourse.bacc as bacc
import concourse.tile as tile
from concourse import mybir
from concourse.bass_utils import run_bass_kernel_spmd

F32 = mybir.dt.float32
BF16 = mybir.dt.bfloat16
F8 = mybir.dt.float8e4
AF = mybir.ActivationFunctionType

B, N, W, R = 16, 32768, 128, 8
NCORES = 8
BLOC = B // NCORES          # batches per core
T = N // 128                # 256 n-tiles of 128 per batch
NG = 8                      # DMA groups per batch
TPG = T // NG               # 32 tiles per group (4096 n, 1MB bf16)
SUB = 2                     # norm-pipeline sub-slots per group
PIPE_LAG = 1                # sub-slots the norm matmuls trail the sims by
TPS = TPG // SUB            # 16 tiles per sub-slot

# ---- tuning knobs ----
# engine squaring each half-group (cycled): "v"=DVE, "a"=ACT
SQUARE_ENGINES = "av"
# DMA queue per group (cycled): "s"=sync, "g"=gpsimd
DMA_QUEUES = "s"
EARLY_G = NG - 3            # groups [0..EARLY_G] exponentiated early
OUT_SPLIT = 2               # final scale+store chunks (tail overlap)
CH = T // OUT_SPLIT


def build_program():
    nc = bacc.Bacc("TRN2", target_bir_lowering=False, debug=False, num_devices=NCORES)

    memT = nc.dram_tensor("memT", [BLOC, W, N], BF16, kind="ExternalInput").ap()
    rv = nc.dram_tensor("read_vectors", [BLOC, W, R], F32, kind="ExternalInput").ap()
    rs = nc.dram_tensor("read_strengths", [BLOC, R], F32, kind="ExternalInput").ap()
    ones = nc.dram_tensor("ones", [128, 128], F32, kind="ExternalInput").ap()
    out = nc.dram_tensor(
        "out", [BLOC, OUT_SPLIT, 128, R, CH], BF16, kind="ExternalOutput"
    ).ap()

    with ExitStack() as ctx:
        tc = ctx.enter_context(tile.TileContext(nc))

        const_pool = ctx.enter_context(tc.tile_pool(name="const", bufs=1))
        ones_t = const_pool.tile([128, 128], F32)
        nc.scalar.dma_start(ones_t[:], ones)
        ones1_bf = const_pool.tile([128, 1], BF16)
        nc.vector.tensor_copy(ones1_bf[:], ones_t[:, 0:1])
        # warm the SQRT act table while the first DMAs stream
        sqrt_warm = const_pool.tile([128, 1], F32)
        nc.scalar.activation(sqrt_warm[:], ones_t[:, 0:1], AF.Sqrt)

        in_pool = ctx.enter_context(tc.tile_pool(name="mem_in", bufs=6))
        sq_pool = ctx.enter_context(tc.tile_pool(name="sq", bufs=4))
        scps_pool = ctx.enter_context(tc.tile_pool(name="scps", bufs=4, space="PSUM"))
        nrps_pool = ctx.enter_context(tc.tile_pool(name="nrps", bufs=3, space="PSUM"))
        prep_pool = ctx.enter_context(tc.tile_pool(name="prep", bufs=1, space="PSUM"))
        smalls = ctx.enter_context(tc.tile_pool(name="smalls", bufs=3))
        rvp_pool = ctx.enter_context(tc.tile_pool(name="rvps", bufs=1))
        score_pool = ctx.enter_context(tc.tile_pool(name="scores", bufs=2))
        outbf_pool = ctx.enter_context(tc.tile_pool(name="outbf", bufs=2))

        # ---- read-vector prep for both batches: rv' = rv*strength/||rv|| ----
        rvp_bfs = []
        for b in range(BLOC):
            rv_t = smalls.tile([128, R], F32)
            nc.scalar.dma_start(rv_t[:], rv[b])
            rs_t = smalls.tile([1, R], F32)
            nc.scalar.dma_start(rs_t[:], rs[b : b + 1, :])

            rv2 = smalls.tile([128, R], F32)
            nc.vector.tensor_mul(rv2[:], rv_t[:], rv_t[:])
            nv2_ps = prep_pool.tile([128, R], F32, tag="prep")
            nc.tensor.matmul(nv2_ps[:], ones_t[:], rv2[:], start=True, stop=True)
            rnv = smalls.tile([128, R], F32)
            nc.vector.reciprocal(rnv[:], nv2_ps[:])
            inv_nv = smalls.tile([128, R], F32)
            nc.scalar.activation(inv_nv[:], rnv[:], AF.Sqrt)
            rsb_ps = prep_pool.tile([128, R], F32, tag="prep")
            nc.tensor.matmul(
                rsb_ps[:], ones_t[0:1, :], rs_t[:], start=True, stop=True
            )
            factor = smalls.tile([128, R], F32)
            nc.vector.tensor_mul(factor[:], rsb_ps[:], inv_nv[:])
            rvp = smalls.tile([128, R], F32, tag="rvp")
            nc.vector.tensor_mul(rvp[:], rv_t[:], factor[:])
            rvp_bf = rvp_pool.tile([128, R], BF16, tag=f"rvpbf{b}")
            nc.vector.tensor_copy(rvp_bf[:], rvp[:])
            rvp_bfs.append(rvp_bf)

        sq_i = 0
        pair_scps = {}
        subs = []  # pipeline: (g, s, sq_g, scps, nrps, scores, s1a_holder)

        def issue_norm_sub(ent):
            g, s, sq_g, scps, nrps, scores, s1a = ent
            half = (g % 2) * TPG
            for j in range(s * TPS, (s + 1) * TPS):
                nc.tensor.matmul(
                    nrps[:, half + j : half + j + 1],
                    sq_g[:, j * 128 : (j + 1) * 128],
                    ones1_bf[:],
                    start=True,
                    stop=True,
                )
            if s < SUB - 1 or g % 2 == 0:
                return
            # pair complete: one Sqrt+reciprocal for both groups, two muls
            nrm_g = smalls.tile([128, 2 * TPG], F32, tag="nrm")
            nc.scalar.activation(nrm_g[:], nrps[:], AF.Sqrt)
            inv_nrm = smalls.tile([128, 2 * TPG], F32, tag="invnrm")
            nc.vector.reciprocal(inv_nrm[:], nrm_g[:])
            for gg in (g - 1, g):
                hh = (gg % 2) * TPG
                nc.vector.tensor_mul(
                    scores[:, :, gg * TPG : (gg + 1) * TPG],
                    pair_scps[gg][:]
                    .rearrange("p (t r) -> p t r", r=R)
                    .transpose([0, 2, 1]),
                    inv_nrm[:, hh : hh + TPG]
                    .unsqueeze(1)
                    .broadcast_to([128, R, TPG]),
                )
            if g == EARLY_G:
                hi = (EARLY_G + 1) * TPG
                nc.scalar.activation(
                    scores[:, :, :hi], scores[:, :, :hi], AF.Exp
                )
                nc.vector.reduce_sum(
                    s1a[:], scores[:, :, :hi], axis=mybir.AxisListType.X
                )

        for b in range(BLOC):
            scores = score_pool.tile([128, R, T], F32)
            s1a = smalls.tile([128, R], F32, tag="s1a")
            rvp_bf = rvp_bfs[b]

            for g in range(NG):
                mem_g = in_pool.tile([128, TPG * 128], BF16)
                qe = DMA_QUEUES[g % len(DMA_QUEUES)]
                src = memT[b, :, g * TPG * 128 : (g + 1) * TPG * 128]
                if qe == "g":
                    nc.gpsimd.dma_start(mem_g[:], src)
                else:
                    nc.sync.dma_start(mem_g[:], src)

                # squares for row norms, issued per half-group
                sq_g = sq_pool.tile([128, TPG * 128], BF16)
                scps = scps_pool.tile([128, TPG * R], F32)
                if g % 2 == 0:
                    nrps = nrps_pool.tile([128, 2 * TPG], F32)
                pair_scps[g] = scps
                for s in range(SUB):
                    ssl = slice(s * TPS * 128, (s + 1) * TPS * 128)
                    se = SQUARE_ENGINES[sq_i % len(SQUARE_ENGINES)]
                    sq_i += 1
                    if se == "a":
                        nc.scalar.square(sq_g[:, ssl], mem_g[:, ssl])
                    elif se == "g":
                        nc.gpsimd.tensor_mul(
                            sq_g[:, ssl], mem_g[:, ssl], mem_g[:, ssl]
                        )
                    else:
                        nc.vector.tensor_mul(sq_g[:, ssl], mem_g[:, ssl], mem_g[:, ssl])

                for s in range(SUB):
                    for j in range(s * TPS, (s + 1) * TPS):
                        nc.tensor.matmul(
                            scps[:, j * R : (j + 1) * R],
                            mem_g[:, j * 128 : (j + 1) * 128],
                            rvp_bf[:],
                            start=True,
                            stop=True,
                        )
                    subs.append((g, s, sq_g, scps, nrps, scores, s1a))
                    if len(subs) > PIPE_LAG:
                        issue_norm_sub(subs.pop(0))

            # flush before this batch's softmax tail reads `scores`
            while subs:
                issue_norm_sub(subs.pop(0))

            # ---- softmax tail (groups EARLY_G+1 .. NG-1) ----
            lo = (EARLY_G + 1) * TPG
            nc.scalar.activation(scores[:, :, lo:], scores[:, :, lo:], AF.Exp)
            s1 = smalls.tile([128, R], F32, tag="s1")
            nc.vector.reduce_sum(s1[:], scores[:, :, lo:], axis=mybir.AxisListType.X)
            if EARLY_G >= 0:
                nc.vector.tensor_add(s1[:], s1[:], s1a[:])
            tot_ps = prep_pool.tile([128, R], F32, tag="prep")
            nc.tensor.matmul(tot_ps[:], ones_t[:], s1[:], start=True, stop=True)
            inv_tot = smalls.tile([128, R], F32)
            nc.vector.reciprocal(inv_tot[:], tot_ps[:])
            for o in range(OUT_SPLIT):
                sl = slice(o * CH, (o + 1) * CH)
                ob = outbf_pool.tile([128, R, CH], BF16)
                nc.vector.tensor_mul(
                    ob[:],
                    scores[:, :, sl],
                    inv_tot[:].unsqueeze(2).broadcast_to([128, R, CH]),
                )
                nc.scalar.dma_start(out[b, o], ob[:])

    nc.compile()
    return nc


_program = None
last_results = None


def _get_program():
    global _program
    if _program is None:
        _program = build_program()
    return _program


def kernel(memory, read_strengths, read_vectors):
    memory = np.asarray(memory, dtype=np.float32)
    read_strengths = np.asarray(read_strengths, dtype=np.float32)
    read_vectors = np.asarray(read_vectors, dtype=np.float32)

    nc = _get_program()
    ones_m = np.ones((128, 128), dtype=np.float32)
    in_maps = []
    for c in range(NCORES):
        sl = slice(c * BLOC, (c + 1) * BLOC)
        memT = np.ascontiguousarray(memory[sl].transpose(0, 2, 1)).astype(
            ml_dtypes.bfloat16
        )
        in_maps.append(
            {
                "memT": memT,
                "read_vectors": np.ascontiguousarray(read_vectors[sl]),
                "read_strengths": np.ascontiguousarray(read_strengths[sl]),
                "ones": ones_m,
            }
        )

    global last_results
    last_results = run_bass_kernel_spmd(nc, in_maps, list(range(NCORES)))
    res = last_results.results
    outs = []
    for c in range(NCORES):
        o = np.asarray(res[c]["out"]).astype(np.float32)
        # (BLOC, OUT_SPLIT, 128, R, CH); n = (o*CH + t')*128 + p
        outs.append(o.transpose(0, 1, 4, 2, 3).reshape(BLOC, N, R))
    return np.concatenate(outs, axis=0)
